# revision 14
# baseline (speedup 1.0000x reference)
"""Trainium2 Bass kernel for nn_Gate_Net (sigmoid gate cumprod over doc windows).

Math per doc (L=128 sentences-1, K=127 window offsets), scores s[129]:
  f = s[:128], b = s[1:129]
  fwd_gate[j,k] = sigmoid(100*(f[j-k] - f[j]) + 5)   (f[j-k]=0 if j<k)
  bwd_gate[j,k] = sigmoid(100*(b[j+k+1] - b[j]) + 5) (b[j+k+1]=0 if j+k+1>=128)
  out = stack([cumprod_k fwd_gate, cumprod_k bwd_gate])  -> [2, N, 128, 127]

Device strategy (per core, 256 docs in 2 blocks of 128):
  - gather: HW indirect DMA consumes its offset AP partition-fastest and
    writes 1-element descriptors sequentially into partition 0's row when
    the dest AP is [128, N, 1].  Six chunked gathers (spread across 4 SWDGE
    queues for parallel descriptor drain) stream all 33024 score[idx]
    values into partition-0 rows, bounce through a DRAM scratch laid out
    as s~T[blk][129][128], then reload as F^T = rows 0..127 / B^T = rows
    1..128.  The host pre-permutes the index stream to match.
  - split fp32 -> bf16 hi/mid (2 splits ~ 16-bit mantissa; arg err ~6e-3
    -> sigmoid err ~1.5e-3, well under the 2e-2 gate)
  - arg[d,(j,k)] = sum_p F[p,d] * W[p,(j,k)] with host-built constant
      W_fwd[p,(j,k)] = [p==j-k] - [p==j],  W_bwd[p,(j,k)] = [p==j+k+1] - [p==j]
    as 2 accumulating bf16 matmuls per 508-col PSUM window
  - ACT: gate = sigmoid(100*psum + 5) -> FP32 (fp32 gates keep the long
    constant-gate tail products exact; bf16 gates fail the 2e-2 bar)
  - DVE tensor_tensor_scan(op0=max, data0=mask(1 at k%127==0), op1=mult,
    data1=gate fp32, out bf16): fp32 scan state, one output rounding.
    Stripes are 2032 = 16*127 so every segment is stripe-local.
  - bf16 output stripes DMA'd to HBM; host casts back to f32
"""

import sys

sys.path.insert(0, "/opt/trn_rl_repo")

import numpy as np
import ml_dtypes

import concourse.bacc as bacc
import concourse.bass as bass
import concourse.tile as tile
from concourse import mybir
from concourse.bass_utils import run_bass_kernel_spmd

N_CORES = 8
POOL = 300000
N_DOCS = 2048
DOC_LEN = 129
L = DOC_LEN - 1          # 128
K = L - 1                # 127
JK = L * K               # 16256
DOCS_PER_CORE = N_DOCS // N_CORES  # 256
BLOCKS = DOCS_PER_CORE // 128      # 2
STRIPE = 16 * K          # 2032
N_STRIPES = JK // STRIPE           # 8
MM_WIN = STRIPE // 4               # 508
GROWS = 43               # scratch rows per gather chunk (129 = 3*43)
GCH = GROWS * 128        # 5504 elements per chunk
NCH = DOC_LEN // GROWS   # 3 chunks per block
GCOLS = GCH // 128       # 43 idx cols per chunk

_BF16 = ml_dtypes.bfloat16


def _build_consts():
    j = np.arange(L)[:, None]   # [128,1]
    k = np.arange(K)[None, :]   # [1,127]
    p = np.arange(128)[:, None, None]
    w_fwd = ((j[None] - k[None]) == p).astype(np.float32) - (
        (j[None] == p) & np.ones_like(k[None], bool)
    ).astype(np.float32)
    w_bwd = ((j[None] + k[None] + 1) == p).astype(np.float32) - (
        (j[None] == p) & np.ones_like(k[None], bool)
    ).astype(np.float32)
    w_fwd = w_fwd.reshape(128, JK).astype(_BF16)
    w_bwd = w_bwd.reshape(128, JK).astype(_BF16)
    mask = np.zeros((128, STRIPE), dtype=_BF16)
    mask[:, ::K] = 1.0
    return w_fwd, w_bwd, mask


def build_program():
    nc = bacc.Bacc("TRN2", target_bir_lowering=False, debug=False)
    f32 = mybir.dt.float32
    bf16 = mybir.dt.bfloat16

    score_d = nc.dram_tensor("score", [POOL, 1], f32, kind="ExternalInput")
    gidx_d = nc.dram_tensor("gidx", [128, 2 * NCH * GCOLS, 1],
                            mybir.dt.int32, kind="ExternalInput")
    wf_d = nc.dram_tensor("w_fwd", [128, JK], bf16, kind="ExternalInput")
    wb_d = nc.dram_tensor("w_bwd", [128, JK], bf16, kind="ExternalInput")
    mask_d = nc.dram_tensor("mask", [128, STRIPE], bf16, kind="ExternalInput")
    out_d = nc.dram_tensor("out", [2, DOCS_PER_CORE, JK], bf16,
                           kind="ExternalOutput")

    with tile.TileContext(nc) as tc:
        with (
            tc.tile_pool(name="consts", bufs=1) as consts,
            tc.tile_pool(name="prep", bufs=4) as prep,
            tc.tile_pool(name="gth", bufs=4) as gpool,
            tc.tile_pool(name="gates", bufs=3) as gates,
            tc.tile_pool(name="outs", bufs=3) as outs,
            tc.tile_pool(name="psum", bufs=2, space="PSUM") as psum,
            tc.tile_pool(name="dram", bufs=1, space="DRAM") as dpool,
        ):
            # ---- constants first on the sync ring (never gather-gated) ----
            gidx = consts.tile([128, 2 * NCH * GCOLS, 1], mybir.dt.int32)
            nc.sync.dma_start(gidx[:], gidx_d[:])
            w_sb = {}
            for dname, dram in (("f", wf_d), ("b", wb_d)):
                wt = consts.tile([128, JK], bf16, tag=f"w_{dname}")
                for c0 in range(0, JK, STRIPE):
                    nc.sync.dma_start(wt[:, c0:c0 + STRIPE],
                                      dram[:, c0:c0 + STRIPE])
                w_sb[dname] = wt
            mask = consts.tile([128, STRIPE], bf16)
            nc.sync.dma_start(mask[:], mask_d[:])
            bias5 = consts.tile([128, 1], f32)
            nc.gpsimd.memset(bias5[:], 5.0)

            # ---- gather (descriptor drain is the long pole); each block's
            # ft loads + splits issue right after its own scratch writes so
            # they sit ahead of the next block's writes in the sync FIFO ----
            scratch = dpool.tile([BLOCKS, DOC_LEN, 128], f32)
            splits = {}
            for blk in range(BLOCKS):
                for ck in range(NCH):
                    c0 = (blk * NCH + ck) * GCOLS
                    gt = gpool.tile([128, GCH, 1], f32, tag="g")
                    nc.gpsimd.indirect_dma_start(
                        out=gt[:],
                        out_offset=None,
                        in_=score_d[:],
                        in_offset=bass.IndirectOffsetOnAxis(
                            ap=gidx[:, c0:c0 + GCOLS, :], axis=0),
                    )
                    nc.sync.dma_start(
                        scratch[blk, ck * GROWS:(ck + 1) * GROWS, :],
                        gt[0:1, :, 0])
                for dname, r0 in (("f", 0), ("b", 1)):
                    ft = prep.tile([128, 128], f32, tag="ft")
                    nc.sync.dma_start(ft[:], scratch[blk, r0:r0 + 128, :])
                    hi = consts.tile([128, 128], bf16, tag=f"hi{dname}{blk}")
                    nc.vector.tensor_copy(hi[:], ft[:])
                    hi32 = prep.tile([128, 128], f32, tag="t32")
                    nc.vector.tensor_copy(hi32[:], hi[:])
                    r1 = prep.tile([128, 128], f32, tag="t32")
                    nc.vector.tensor_sub(r1[:], ft[:], hi32[:])
                    mid = consts.tile([128, 128], bf16, tag=f"mid{dname}{blk}")
                    nc.vector.tensor_copy(mid[:], r1[:])
                    splits[(dname, blk)] = [hi, mid]

            # ---- main pipeline ----
            for blk in range(BLOCKS):
                d0 = blk * 128
                for dname in ("f", "b"):
                    wt = w_sb[dname]
                    sp = splits[(dname, blk)]
                    di = 0 if dname == "f" else 1
                    for s in range(N_STRIPES):
                        c0 = s * STRIPE
                        ps = psum.tile([128, STRIPE], f32, tag="mm")
                        # windows must be PSUM-bank aligned (512 f32): a
                        # straddling window's start-clear races the previous
                        # window's draining writes in the shared bank
                        for w0 in range(0, STRIPE, 512):
                            wl = min(512, STRIPE - w0)
                            for si in range(2):
                                nc.tensor.matmul(
                                    ps[:, w0:w0 + wl],
                                    sp[si][:],
                                    wt[:, c0 + w0:c0 + w0 + wl],
                                    start=(si == 0),
                                    stop=(si == 1),
                                )
                        gate = gates.tile([128, STRIPE], f32)
                        nc.scalar.activation(
                            gate[:], ps[:],
                            mybir.ActivationFunctionType.Sigmoid,
                            bias=bias5[:], scale=100.0,
                        )
                        ot = outs.tile([128, STRIPE], bf16)
                        nc.vector.tensor_tensor_scan(
                            out=ot[:],
                            data0=mask[:],
                            data1=gate[:],
                            initial=0.0,
                            op0=mybir.AluOpType.max,
                            op1=mybir.AluOpType.mult,
                        )
                        nc.sync.dma_start(
                            out_d[di, d0:d0 + 128, c0:c0 + STRIPE], ot[:])

    nc.compile()
    return nc


_NC = None


def _get_nc():
    global _NC
    if _NC is None:
        _NC = build_program()
    return _NC


def _in_maps(score, idx):
    """score: [POOL] f32, idx: [N_DOCS, DOC_LEN] int32 -> per-core inputs."""
    w_fwd, w_bwd, mask = _build_consts()
    score2d = score.reshape(POOL, 1)
    maps = []
    for c in range(N_CORES):
        sl = idx[c * DOCS_PER_CORE:(c + 1) * DOCS_PER_CORE]  # [256, 129]
        gidx = np.empty((128, 2 * NCH * GCOLS), dtype=np.int32)
        for blk in range(BLOCKS):
            # stream for this block: s~T[t, d] row-major, t=0..129, d block docs
            stream = np.ascontiguousarray(
                sl[blk * 128:(blk + 1) * 128].T).ravel()
            for ck in range(NCH):
                c0 = (blk * NCH + ck) * GCOLS
                gidx[:, c0:c0 + GCOLS] = (
                    stream[ck * GCH:(ck + 1) * GCH].reshape(GCOLS, 128).T)
        maps.append({
            "score": score2d,
            "gidx": gidx.reshape(128, 2 * NCH * GCOLS, 1),
            "w_fwd": w_fwd,
            "w_bwd": w_bwd,
            "mask": mask,
        })
    return maps


def _post(raw):
    """[2, D, JK] bf16 -> [2, D, L, K] f32"""
    return np.asarray(raw).astype(np.float32).reshape(2, DOCS_PER_CORE, L, K)


def kernel(score, score_idx):
    score = np.ascontiguousarray(np.asarray(score, dtype=np.float32))
    idx = np.ascontiguousarray(np.asarray(score_idx).astype(np.int32))
    assert score.shape == (POOL,) and idx.shape == (N_DOCS, DOC_LEN)

    nc = _get_nc()
    res = run_bass_kernel_spmd(nc, _in_maps(score, idx),
                               core_ids=list(range(N_CORES)))
    shards = [_post(r["out"]) for r in res.results]
    return np.concatenate(shards, axis=1)


if __name__ == "__main__":
    rng = np.random.default_rng(0)
    score = rng.standard_normal(POOL).astype(np.float32)
    idx = rng.integers(0, POOL, size=(N_DOCS, DOC_LEN)).astype(np.int32)
    out = kernel(score, idx)
    print(out.shape, out.dtype, float(out[0, 0, :4, :4].sum()))
